# revision 16
# baseline (speedup 1.0000x reference)
"""Fused bmm + residual kernel for Trainium2 (8 NeuronCores, data-parallel).

out[n,c,p] = x[n,c,p] + alpha * sum_q attn[n,p,q] * D[n,q,c]
  N=2048, C=512, H=W=7 (HW=49)

Sharding: batch N across 8 cores (256 each), no collectives.

Scheme (tolerance is 2e-2; measured max rel err ~1.1e-2):
 - Host pre-packs all tensors to device layouts, so every DMA is a
   contiguous partition-major block with multi-KB descriptors.
 - attn is transposed on the host (alpha folded in); no device transposes.
 - Pair packing with K=98, gap-free: even batch at partitions 0:49, odd
   at 49:98 (matmul operand partition base MUST be 0 here: bases 32/64
   pass bass asserts but base-64 aborts on this runtime).
 - rhs [98, (2, 49)] is block-diagonal, off-diag zeroed once per ring
   tile; lhsT = D pair [98, 512] in c-chunks of 128; 4 MMs + 1 add /pair.
 - D loads as ONE 98-row DMA per group: HWDGE spreads a DMA over
   min(16, ceil(rows/7)) SDMA engines by relative row, so 98 rows ride
   14 engines (49-row DMAs would pile onto 7).  Per-engine ~21.5 GB/s is
   the roofline; busiest-engine bytes set the kernel time.
 - x loads in fp8-e4m3: its rounding error is additive (~0.28 abs vs the
   ~0.51 abs tolerance budget) and does not pass through the contraction,
   so fp8 is safe for x but NOT for D/attn.  D/attn/out use bf16.
 - PSUM y [128, 4, 2, (2, 49)] fp32 covers TWO pairs; one DVE add per 2
   pairs (FD=784 amortizes the ~120-cycle DVE op startup).
 - Load/store DMAs alternate between the two HWDGE rings per group.
 - Variable group sizes: small warmup groups start the matmuls early,
   small tail groups shorten the drain.
"""
import sys

sys.path.insert(0, "/opt/trn_rl_repo")

import numpy as np
import ml_dtypes

BF16 = ml_dtypes.bfloat16
FP8 = ml_dtypes.float8_e4m3fn

# ---- static problem config (hardcoded per harness contract) ----
N_TOT, C, HW = 2048, 512, 49
N_CORES = 8
NB = N_TOT // N_CORES        # 256 batches per core
NPAIRS = NB // 2             # 128 pairs per core
GMAX = 16                    # max batches per group
GROUPS = [4, 4, 8] + [16] * 14 + [8, 4, 4]   # batches per group, sum=256
assert sum(GROUPS) == NB and all(g % 4 == 0 for g in GROUPS)
NAT = 4                      # rhs (attn^T) ring size

_cached = {}


def _build_bass():
    import concourse.bacc as bacc
    import concourse.mybir as mybir
    from concourse import tile

    bf16 = mybir.dt.bfloat16
    fp8 = mybir.dt.float8e4
    f32 = mybir.dt.float32
    nc = bacc.Bacc(None, target_bir_lowering=False)

    x_d = nc.dram_tensor("xp", [128, NB, 4, HW], fp8, kind="ExternalInput")
    ae_d = nc.dram_tensor("ae", [HW, NPAIRS, HW], bf16, kind="ExternalInput")
    ao_d = nc.dram_tensor("ao", [HW, NPAIRS, HW], bf16, kind="ExternalInput")
    d_d = nc.dram_tensor("dp", [2 * HW, NPAIRS, C], bf16, kind="ExternalInput")
    o_d = nc.dram_tensor("op", [128, NB, 4, HW], bf16, kind="ExternalOutput")

    with tile.TileContext(nc) as tc:
        with (
            tc.tile_pool(name="atp", bufs=NAT) as atp,
            tc.tile_pool(name="dp", bufs=4) as dp,
            tc.tile_pool(name="xp", bufs=5) as xp,
            tc.tile_pool(name="op", bufs=5) as op,
            tc.tile_pool(name="yp", bufs=4, space="PSUM") as yp,
        ):
            # rhs ring: block-diagonal [98, (2, 49)] per pair; off-diagonal
            # blocks must stay zero, so memset each ring tile once up front.
            at_tiles = []
            for i in range(NAT):
                t = atp.tile([2 * HW, 2, GMAX // 2, HW], bf16, tag="at")
                nc.vector.memset(t, 0.0)
                at_tiles.append(t)

            b0 = 0
            for g, gsz in enumerate(GROUPS):
                npair = gsz // 2
                i0 = b0 // 2
                ld = nc.sync if g % 2 == 0 else nc.scalar
                st = nc.scalar if g % 2 == 0 else nc.sync

                d_t = dp.tile([2 * HW, GMAX // 2, C], bf16, tag="d")
                ld.dma_start(out=d_t[:, 0:npair], in_=d_d[:, i0:i0 + npair])

                at_t = at_tiles[g % NAT]
                ld.dma_start(out=at_t[0:HW, 0, 0:npair, :],
                             in_=ae_d[:, i0:i0 + npair])
                ld.dma_start(out=at_t[HW:2 * HW, 1, 0:npair, :],
                             in_=ao_d[:, i0:i0 + npair])

                x_t = xp.tile([128, GMAX, 4, HW], fp8, tag="x")
                st.dma_start(out=x_t[:, 0:gsz], in_=x_d[:, b0:b0 + gsz])

                o_t = op.tile([128, GMAX, 4, HW], bf16, tag="o")

                for ii in range(npair // 2):
                    # y covers two pairs: free dims (j, u, (b, p))
                    y_ps = yp.tile([128, 4, 2, 2 * HW], f32, tag="y")
                    for u in range(2):
                        i = 2 * ii + u
                        for j in range(4):
                            nc.tensor.matmul(
                                out=y_ps[:, j, u, :],
                                lhsT=d_t[0:2 * HW, i, 128 * j:128 * (j + 1)],
                                rhs=at_t[0:2 * HW, :, i, :],
                                start=True,
                                stop=True,
                            )
                    # regroup (j, u, b, p) -> (u, b, j, p) = (n, j, p)
                    y4 = y_ps.rearrange("r j u (b p) -> r (u b) j p", b=2)
                    nc.vector.tensor_add(
                        out=o_t[:, 4 * ii:4 * ii + 4, :, :],
                        in0=y4,
                        in1=x_t[:, 4 * ii:4 * ii + 4, :, :],
                    )

                st.dma_start(out=o_d[:, b0:b0 + gsz], in_=o_t[:, 0:gsz])
                b0 += gsz

    nc.finalize()
    return nc


def _get_nc():
    if "nc" not in _cached:
        _cached["nc"] = _build_bass()
    return _cached["nc"]


def _in_maps(x, attn, D, alpha):
    a0 = np.float32(np.asarray(alpha).reshape(-1)[0])

    # x[n, c, p] -> xp[core, r, n, j, p] with c = 128j + r
    xr = np.asarray(x, dtype=np.float32).reshape(N_CORES, NB, 4, 128, HW)
    xp = np.ascontiguousarray(xr.transpose(0, 3, 1, 2, 4)).astype(FP8)

    # attn[n, p, q] * alpha -> attn_T[n, q, p] -> [core, q, i, p] even/odd
    at = (np.asarray(attn, dtype=np.float32) * a0).transpose(0, 2, 1)
    at = at.reshape(N_CORES, NPAIRS, 2, HW, HW)
    ae = np.ascontiguousarray(at[:, :, 0].transpose(0, 2, 1, 3)).astype(BF16)
    ao = np.ascontiguousarray(at[:, :, 1].transpose(0, 2, 1, 3)).astype(BF16)

    # D[n, q, c] -> [core, b*49+q, i, c] (even rows 0:49, odd rows 49:98)
    dr = np.asarray(D, dtype=np.float32).reshape(N_CORES, NPAIRS, 2, HW, C)
    dp = np.ascontiguousarray(dr.transpose(0, 2, 3, 1, 4)).astype(BF16)
    dp = dp.reshape(N_CORES, 2 * HW, NPAIRS, C)

    return [
        {"xp": xp[c], "ae": ae[c], "ao": ao[c], "dp": dp[c]}
        for c in range(N_CORES)
    ]


def kernel(x: np.ndarray, attn: np.ndarray, D: np.ndarray, alpha: np.ndarray) -> np.ndarray:
    from concourse import bass_utils

    nc = _get_nc()
    res = bass_utils.run_bass_kernel_spmd(
        nc, _in_maps(x, attn, D, alpha), core_ids=list(range(N_CORES))
    )
    # op[r, n, j, p] -> out[n, c, p] with c = 128j + r
    out = np.stack([res.results[c]["op"] for c in range(N_CORES)])
    out = out.astype(np.float32).transpose(0, 2, 3, 1, 4)
    return np.ascontiguousarray(out).reshape(N_TOT, C, 7, 7)


# revision 17
# speedup vs baseline: 1.0195x; 1.0195x over previous
"""Fused bmm + residual kernel for Trainium2 (8 NeuronCores, data-parallel).

out[n,c,p] = x[n,c,p] + alpha * sum_q attn[n,p,q] * D[n,q,c]
  N=2048, C=512, H=W=7 (HW=49)

Sharding: batch N across 8 cores (256 each), no collectives.

Scheme (tolerance is 2e-2; measured max rel err ~1.1e-2):
 - Host pre-packs all tensors to device layouts, so every DMA is a
   contiguous partition-major block with multi-KB descriptors.
 - attn is transposed on the host (alpha folded in); no device transposes.
 - Pair packing with K=98, gap-free: even batch at partitions 0:49, odd
   at 49:98 (matmul operand partition base MUST be 0 here: bases 32/64
   pass bass asserts but base-64 aborts on this runtime).
 - rhs [98, (2, 49)] is block-diagonal, off-diag zeroed once per ring
   tile; lhsT = D pair [98, 512] in c-chunks of 128; 4 MMs + 1 add /pair.
 - D loads as ONE 98-row DMA per group: HWDGE spreads a DMA over
   min(16, ceil(rows/7)) SDMA engines by relative row, so 98 rows ride
   14 engines (49-row DMAs would pile onto 7).  Per-engine ~21.5 GB/s is
   the roofline; busiest-engine bytes set the kernel time.
 - x loads in fp8-e4m3: its rounding error is additive (~0.28 abs vs the
   ~0.51 abs tolerance budget) and does not pass through the contraction,
   so fp8 is safe for x but NOT for D/attn.  D/attn/out use bf16.
 - PSUM y [128, 4, 2, (2, 49)] fp32 covers TWO pairs; one DVE add per 2
   pairs (FD=784 amortizes the ~120-cycle DVE op startup).
 - Load/store DMAs alternate between the two HWDGE rings per group.
 - Variable group sizes: small warmup groups start the matmuls early,
   small tail groups shorten the drain.
"""
import sys

sys.path.insert(0, "/opt/trn_rl_repo")

import numpy as np
import ml_dtypes

BF16 = ml_dtypes.bfloat16
FP8 = ml_dtypes.float8_e4m3fn

# ---- static problem config (hardcoded per harness contract) ----
N_TOT, C, HW = 2048, 512, 49
N_CORES = 8
NB = N_TOT // N_CORES        # 256 batches per core
NPAIRS = NB // 2             # 128 pairs per core
GMAX = 16                    # max batches per group
GROUPS = [4, 4, 8] + [16] * 14 + [8, 4, 4]   # batches per group, sum=256
assert sum(GROUPS) == NB and all(g % 4 == 0 for g in GROUPS)
NAT = 4                      # rhs (attn^T) ring size

_cached = {}


def _build_bass():
    import concourse.bacc as bacc
    import concourse.mybir as mybir
    from concourse import tile

    bf16 = mybir.dt.bfloat16
    fp8 = mybir.dt.float8e4
    f32 = mybir.dt.float32
    nc = bacc.Bacc(None, target_bir_lowering=False)

    x_d = nc.dram_tensor("xp", [128, NB, 4, HW], fp8, kind="ExternalInput")
    ao_d = nc.dram_tensor("ao", [HW, NPAIRS, HW], bf16, kind="ExternalInput")
    cb_d = nc.dram_tensor("cb", [2 * HW, NPAIRS, C + HW], bf16, kind="ExternalInput")
    o_d = nc.dram_tensor("op", [128, NB, 4, HW], bf16, kind="ExternalOutput")

    with tile.TileContext(nc) as tc:
        with (
            tc.tile_pool(name="atp", bufs=NAT) as atp,
            tc.tile_pool(name="dp", bufs=4) as dp,
            tc.tile_pool(name="xp", bufs=5) as xp,
            tc.tile_pool(name="op", bufs=5) as op,
            tc.tile_pool(name="yp", bufs=4, space="PSUM") as yp,
        ):
            # rhs ring: block-diagonal [98, (2, 49)] per pair; off-diagonal
            # blocks must stay zero, so memset each ring tile once up front.
            at_tiles = []
            for i in range(NAT):
                t = atp.tile([2 * HW, 2, GMAX // 2, HW], bf16, tag="at")
                nc.vector.memset(t, 0.0)
                at_tiles.append(t)

            b0 = 0
            for g, gsz in enumerate(GROUPS):
                npair = gsz // 2
                i0 = b0 // 2
                ld = nc.sync if g % 2 == 0 else nc.scalar
                st = nc.scalar if g % 2 == 0 else nc.sync

                d_t = dp.tile([2 * HW, GMAX // 2, C + HW], bf16, tag="d")
                ld.dma_start(out=d_t[:, 0:npair], in_=cb_d[:, i0:i0 + npair])

                at_t = at_tiles[g % NAT]
                # even diagonal block: base-0 copy on the idle GpSimd engine
                nc.gpsimd.tensor_copy(
                    out=at_t[0:HW, 0, 0:npair, :],
                    in_=d_t[0:HW, 0:npair, C:C + HW],
                )
                ld.dma_start(out=at_t[HW:2 * HW, 1, 0:npair, :],
                             in_=ao_d[:, i0:i0 + npair])

                x_t = xp.tile([128, GMAX, 4, HW], fp8, tag="x")
                st.dma_start(out=x_t[:, 0:gsz], in_=x_d[:, b0:b0 + gsz])

                o_t = op.tile([128, GMAX, 4, HW], bf16, tag="o")

                for ii in range(npair // 2):
                    # y covers two pairs: free dims (j, u, (b, p))
                    y_ps = yp.tile([128, 4, 2, 2 * HW], f32, tag="y")
                    for u in range(2):
                        i = 2 * ii + u
                        for j in range(4):
                            nc.tensor.matmul(
                                out=y_ps[:, j, u, :],
                                lhsT=d_t[0:2 * HW, i, 128 * j:128 * (j + 1)],
                                rhs=at_t[0:2 * HW, :, i, :],
                                start=True,
                                stop=True,
                            )
                    # regroup (j, u, b, p) -> (u, b, j, p) = (n, j, p)
                    y4 = y_ps.rearrange("r j u (b p) -> r (u b) j p", b=2)
                    nc.vector.tensor_add(
                        out=o_t[:, 4 * ii:4 * ii + 4, :, :],
                        in0=y4,
                        in1=x_t[:, 4 * ii:4 * ii + 4, :, :],
                    )

                st.dma_start(out=o_d[:, b0:b0 + gsz], in_=o_t[:, 0:gsz])
                b0 += gsz

    nc.finalize()
    return nc


def _get_nc():
    if "nc" not in _cached:
        _cached["nc"] = _build_bass()
    return _cached["nc"]


def _in_maps(x, attn, D, alpha):
    a0 = np.float32(np.asarray(alpha).reshape(-1)[0])

    # x[n, c, p] -> xp[core, r, n, j, p] with c = 128j + r
    xr = np.asarray(x, dtype=np.float32).reshape(N_CORES, NB, 4, 128, HW)
    xp = np.ascontiguousarray(xr.transpose(0, 3, 1, 2, 4)).astype(FP8)

    # attn[n, p, q] * alpha -> attn_T[n, q, p] -> [core, q, i, p] even/odd
    at = (np.asarray(attn, dtype=np.float32) * a0).transpose(0, 2, 1)
    at = at.reshape(N_CORES, NPAIRS, 2, HW, HW)
    ao = np.ascontiguousarray(at[:, :, 1].transpose(0, 2, 1, 3)).astype(BF16)

    # combined [core, (b,q), i, 561]: cols 0:512 = D rows; even rows carry
    # alpha*A^T_even at cols 512:561; odd rows' attn columns unused (zero)
    dr = np.asarray(D, dtype=np.float32).reshape(N_CORES, NPAIRS, 2, HW, C)
    cb = np.zeros((N_CORES, 2 * HW, NPAIRS, C + HW), dtype=BF16)
    cb[:, 0:HW, :, 0:C] = dr[:, :, 0].transpose(0, 2, 1, 3).astype(BF16)
    cb[:, 0:HW, :, C:] = at[:, :, 0].transpose(0, 2, 1, 3).astype(BF16)
    cb[:, HW:2 * HW, :, 0:C] = dr[:, :, 1].transpose(0, 2, 1, 3).astype(BF16)

    return [
        {"xp": xp[c], "ao": ao[c], "cb": cb[c]}
        for c in range(N_CORES)
    ]


def kernel(x: np.ndarray, attn: np.ndarray, D: np.ndarray, alpha: np.ndarray) -> np.ndarray:
    from concourse import bass_utils

    nc = _get_nc()
    res = bass_utils.run_bass_kernel_spmd(
        nc, _in_maps(x, attn, D, alpha), core_ids=list(range(N_CORES))
    )
    # op[r, n, j, p] -> out[n, c, p] with c = 128j + r
    out = np.stack([res.results[c]["op"] for c in range(N_CORES)])
    out = out.astype(np.float32).transpose(0, 2, 3, 1, 4)
    return np.ascontiguousarray(out).reshape(N_TOT, C, 7, 7)
